# revision 10
# baseline (speedup 1.0000x reference)
"""AttentionBlock kernel for 8 Trainium2 NeuronCores (Bass/Tile).

Reference (per sample):
    q = Wq @ x + bq  [32, N];  k = Wk @ x + bk  [32, N];  v = Wv @ x + bv
    attn = softmax(q^T k, axis=keys)  [N, N],  N = 64*64 = 4096
    out  = gamma * (v @ attn^T) + x

Sharding: data-parallel over batch (16 samples -> 2 per core), weights
replicated on every core; no collectives.

Device algorithm (per sample), chosen so softmax needs no partition-axis
reduction and no transposes; scores are computed transposed ([j, i]) so
the j (key) dim sits on psum partitions:

  - Preprocessing in fp8 (host-quantized x, fused [Wq.T|Wk.T]*16 and
    Wv.T*16): DoubleRow matmuls contract the full K=256 channel dim in
    one instruction; q/k rescaled+biased to bf16, vT tiles rescaled to
    fp8 with an appended column holding 1/gamma (see below).
  - Steady state runs on i-blocks of 256 tokens; per block, 8 "quads"
    (4 j-chunks x 256 i) of scores land in a [128, 1024] psum tile via
    2-way row-tiled concurrent bf16 matmuls (the two concurrent MMs
    write different psum banks - a hard HW rule).  Score psum is triple
    buffered (6 banks) + 2 banks of out accumulation = all 8 banks.
  - The N^2 exp stream is split across TWO engines (the steady-state
    bottleneck).  ScalarE runs native exp (f32 psum -> fp8 sbuf); the
    DVE's share uses a bit trick: uint8 b = round(s*8*log2(e) + 56)
    IS the fp8e4m3 encoding of ~e^s (piecewise-linear mantissa, +-6%
    per element, washes out over the 4096-key softmax; end-to-end rel
    err ~8e-5).  f32->uint8 converts round-to-nearest and saturate at
    0 = exp underflow for free.  Split ~145/111 balances the engines.
  - out^T[i, 0:256] accumulates over j with fp8 DoubleRow matmuls
    (lhsT = e pair [128, 2, 128], rhs = vT_aug pair [128, 2, 257]); psum
    column 256 accumulates sum_j e[j,i]/gamma, so one reciprocal gives
    gamma/denom and one multiply normalizes AND applies gamma.
  - residual add (out += xT, bv pre-folded on host) runs on the idle
    GpSimd engine (SBUF-only operands); output DMAs are contiguous.
"""

import ml_dtypes
import numpy as np

from concourse import bacc, mybir, tile
from concourse.bass_utils import run_bass_kernel_spmd

f32 = mybir.dt.float32
bf16 = mybir.dt.bfloat16
f8 = mybir.dt.float8e4
u8 = mybir.dt.uint8
AF = mybir.ActivationFunctionType
ALU = mybir.AluOpType
DR = mybir.MatmulPerfMode.DoubleRow

B, C, HH, WW = 16, 256, 64, 64
N = HH * WW            # 4096 tokens
CQK = C // 8           # 32
NCORES = 8
S = B // NCORES        # 2 samples per core
CC = C // 128          # 2 channel chunks
NB = N // 512          # 8 blocks of 512 tokens (preproc granularity)
NB2 = N // 256         # 16 i-blocks of 256 tokens (steady state)
NJ = N // 128          # 32 chunks of 128 tokens
JQ = NJ // 4           # 8 quads of 4 j-chunks
ICH = 2                # i-chunks (of 128) per 256-block
WSCALE = 16.0          # fp8 range scaling for the small conv weights
EK = 8 * 1.4426950408889634   # uint8 fast-exp scale: b = s*EK + EC
EC = 56.0
DVE_SHARE = 111        # of 256 exp tiles go to the DVE (uint8 trick)

_PROG_CACHE = {}
last_results = None  # BassKernelResults of the most recent kernel() call
TRACE = False  # set True (e.g. from test.py) to capture an NTFF profile


def _build(gamma: float):
    nc = bacc.Bacc(
        trn_type="TRN2", target_bir_lowering=False, debug=False, num_devices=NCORES
    )
    x_d = nc.dram_tensor("x", [S, C, N], f8, kind="ExternalInput").ap()
    xr_d = nc.dram_tensor("xr", [S, N, C], f32, kind="ExternalInput").ap()
    # wqk = [Wq.T | Wk.T] * WSCALE, wvt = Wv.T * WSCALE (fp8-range scaling)
    wqk_d = nc.dram_tensor("wqk", [C, 2 * CQK], f8, kind="ExternalInput").ap()
    wv_d = nc.dram_tensor("wvt", [C, C], f8, kind="ExternalInput").ap()
    bq_d = nc.dram_tensor("bq", [CQK, 1], f32, kind="ExternalInput").ap()
    bk_d = nc.dram_tensor("bk", [CQK, 1], f32, kind="ExternalInput").ap()
    o_d = nc.dram_tensor("o", [S, N, C], f32, kind="ExternalOutput").ap()

    with tile.TileContext(nc) as tc:
        with tc.tile_pool(name="persist", bufs=1) as pp:
            wqk_sb = pp.tile([128, CC, 2 * CQK], f8, tag="wqk")
            wv_sb = pp.tile([128, CC, C], f8, tag="wv")
            bq_sb = pp.tile([CQK, 1], f32, tag="bq")
            bk_sb = pp.tile([CQK, 1], f32, tag="bk")
            # q on parts 0:32 with a DMA replica on 32:64 (2-way row-tiled
            # score matmuls).  The fused qk psum has k on rows 32:64, so k
            # lands there directly for EVEN j-chunks; odd chunks go through
            # ko staging (parts 32:64) and are DMA-shifted to parts 0:32.
            q_sb = pp.tile([2 * CQK, S, N], bf16, tag="q")
            k_sb = pp.tile([2 * CQK, S, NJ // 2, 128], bf16, tag="k")
            ko_sb = pp.tile([2 * CQK, S, NJ // 2, 128], bf16, tag="ko")
            vt_sb = pp.tile([128, S, NJ, 257], f8, tag="vt")

            for cc in range(CC):
                nc.sync.dma_start(out=wqk_sb[:, cc, :], in_=wqk_d[cc * 128:(cc + 1) * 128, :])
                nc.sync.dma_start(out=wv_sb[:, cc, :], in_=wv_d[cc * 128:(cc + 1) * 128, :])
            nc.sync.dma_start(out=bq_sb, in_=bq_d)
            nc.sync.dma_start(out=bk_sb, in_=bk_d)
            # denominator column of vT_aug carries 1/gamma, so the psum
            # ones-column accumulates D/gamma and its reciprocal is already
            # gamma/D (one multiply normalizes AND applies gamma).
            nc.vector.memset(vt_sb[:, :, :, 256:257], 1.0 / gamma)

            # ---------------- preprocessing: q, k, vT for both samples -------
            with (
                tc.tile_pool(name="xin", bufs=3) as xpool,
                tc.tile_pool(name="pp_qk", bufs=2, space="PSUM") as qkps,
                tc.tile_pool(name="pp_v", bufs=2, space="PSUM") as vps,
            ):
                for s in range(S):
                    xt = xpool.tile([128, CC, N], f8, tag="x")
                    for cc in range(CC):
                        nc.sync.dma_start(
                            out=xt[:, cc, :], in_=x_d[s, cc * 128:(cc + 1) * 128, :]
                        )
                    for nbp in range(NB // 2):
                        # fused q|k DoubleRow matmul over full K=256, psum
                        # [64, 1024]: rows 0:32 = 16*q, rows 32:64 = 16*k
                        pqk = qkps.tile([2 * CQK, 1024], f32, tag="pq")
                        for h2 in range(2):
                            nsl = slice((2 * nbp + h2) * 512, (2 * nbp + h2 + 1) * 512)
                            nc.tensor.matmul(
                                pqk[:, h2 * 512:(h2 + 1) * 512],
                                lhsT=wqk_sb,
                                rhs=xt[:, :, nsl],
                                start=True,
                                stop=True,
                                perf_mode=DR,
                            )
                        # q = psum/WSCALE + bq  (ScalarE, FD=1024)
                        nc.scalar.activation(
                            out=q_sb[0:CQK, s, nbp * 1024:(nbp + 1) * 1024],
                            in_=pqk[0:CQK, :],
                            func=AF.Identity,
                            bias=bq_sb,
                            scale=1.0 / WSCALE,
                        )
                        # k chunks 8*nbp .. 8*nbp+7 -> pair-groups 4*nbp..+3
                        pk8 = pqk[CQK:2 * CQK, :].rearrange("p (a b) -> p a b", b=128)
                        nc.vector.tensor_scalar(
                            k_sb[CQK:2 * CQK, s, 4 * nbp:4 * nbp + 4, :],
                            pk8[:, 0::2, :],
                            1.0 / WSCALE,
                            bk_sb,
                            ALU.mult,
                            ALU.add,
                        )
                        nc.vector.tensor_scalar(
                            ko_sb[CQK:2 * CQK, s, 4 * nbp:4 * nbp + 4, :],
                            pk8[:, 1::2, :],
                            1.0 / WSCALE,
                            bk_sb,
                            ALU.mult,
                            ALU.add,
                        )
                    for njp in range(NJ // 2):
                        # vT pair: psum [128, 512] = two 128-token chunks
                        pv = vps.tile([128, 512], f32, tag="pv")
                        for u in range(2):
                            nsl = slice((2 * njp + u) * 128, (2 * njp + u + 1) * 128)
                            nc.tensor.matmul(
                                pv[:, u * 256:(u + 1) * 256],
                                lhsT=xt[:, :, nsl],
                                rhs=wv_sb,
                                start=True,
                                stop=True,
                                perf_mode=DR,
                            )
                        dst = vt_sb[:, s, 2 * njp:2 * njp + 2, 0:C]
                        pv2 = pv.rearrange("p (a b) -> p a b", b=256)
                        nc.scalar.mul(dst, pv2, 1.0 / WSCALE)
                    # replicate q to partitions 32:64; shift odd-k to parts 0:32
                    nc.sync.dma_start(
                        out=q_sb[CQK:2 * CQK, s, :], in_=q_sb[0:CQK, s, :]
                    )
                    nc.sync.dma_start(
                        out=k_sb[0:CQK, s, :, :], in_=ko_sb[CQK:2 * CQK, s, :, :]
                    )

            # ---------------- steady state: attention ----------------------
            with (
                tc.tile_pool(name="sps", bufs=2, space="PSUM") as sps,
                tc.tile_pool(name="ops", bufs=2, space="PSUM") as ops_pool,
                tc.tile_pool(name="epool", bufs=8) as epool,
                tc.tile_pool(name="xrpool", bufs=10) as xrpool,
                tc.tile_pool(name="otpool", bufs=10) as otpool,
                tc.tile_pool(name="rpool", bufs=8) as rpool,
            ):
                def emit_out_mms(s, ops4, jq, et):
                    # out accumulation for quad jq.  The score/e tile column
                    # layout is [jc0 | jc2 | jc1 | jc3] (each 256 i), so the
                    # rearranged [p, 2, 512] view pairs (jc0, jc1) on cols
                    # 0:256 and (jc2, jc3) on cols 256:512 - matching vt
                    # chunk pairs (4jq, 4jq+1) and (4jq+2, 4jq+3).
                    e4 = et.rearrange("p (u i) -> p u i", u=2)
                    for pr in range(2):
                        for ic in range(ICH):
                            nc.tensor.matmul(
                                ops4[:, ic, 0:257],
                                lhsT=e4[:, :, pr * 256 + ic * 128:
                                        pr * 256 + (ic + 1) * 128],
                                rhs=vt_sb[:, s, 4 * jq + 2 * pr:4 * jq + 2 * pr + 2, :],
                                start=(jq == 0 and pr == 0),
                                stop=(jq == JQ - 1 and pr == 1),
                                perf_mode=DR,
                                skip_group_check=True,
                            )

                def emit_normalize(s, ib, ops4):
                    # Normalize + residual for a finished i-block.  The
                    # psum-reading ops are high-priority (and the two muls go
                    # to DIFFERENT engines) so the out-psum banks free up
                    # immediately after the last out-matmul instead of
                    # queueing behind the engines' exp backlogs -- the next
                    # block's start=True accumulation waits on these reads.
                    i0 = ib * 256
                    xrts, ots = [], []
                    for ic in range(ICH):
                        isl = slice(i0 + ic * 128, i0 + (ic + 1) * 128)
                        xrt = xrpool.tile([128, C], f32, tag="xr")
                        nc.sync.dma_start(out=xrt, in_=xr_d[s, isl, :])
                        xrts.append(xrt)
                    with tc.high_priority(offset=200):
                        rs = rpool.tile([128, ICH, 1], f32, tag="rs")
                        nc.vector.reciprocal(rs, ops4[:, :, 256:257])
                        for ic in range(ICH):
                            ot = otpool.tile([128, C], f32, tag="ot")
                            # ot = psum * (gamma/denom)  (gamma baked in rs)
                            if ic == 0:
                                nc.scalar.mul(ot, ops4[:, ic, 0:C], rs[:, ic, :])
                            else:
                                nc.vector.tensor_scalar_mul(
                                    ot, ops4[:, ic, 0:C], rs[:, ic, :]
                                )
                            ots.append(ot)
                    for ic in range(ICH):
                        isl = slice(i0 + ic * 128, i0 + (ic + 1) * 128)
                        nc.gpsimd.tensor_tensor(
                            ots[ic], ots[ic], xrts[ic], ALU.add
                        )
                        nc.sync.dma_start(out=o_d[s, isl, :], in_=ots[ic])

                # Software pipeline over ALL quads (crossing i-block and
                # sample boundaries): the out-matmuls of quad t are emitted
                # after the scores+exp of quad t+2, so the PE never waits on
                # an in-flight exp and the two exp engines free-run.  The
                # normalize for a block is emitted right after its last
                # out-matmul batch (two quads into the next block), which
                # still precedes the next block's start=True accumulation.
                PIPE = 2
                QT = 0.0009  # virtual per-quad time slot (ms) for the
                # scheduler: pins the score/exp/out interleave so the static
                # schedule can't convoy the PE ahead of the exp engines.
                exp_ctr = 0  # Bresenham split of exp tiles across engines
                pend = []
                ops4 = None
                qi = 0
                for s in range(S):
                    for ib in range(NB2):
                        i0 = ib * 256
                        for jq in range(JQ):
                            qi += 1
                            if jq == 0:
                                ops4 = ops_pool.tile([128, ICH, 512], f32,
                                                     tag="o4", name="ops4")
                            sp = sps.tile([128, 1024], f32, tag="s")
                            # two concurrent row-tiled MM pairs; the pair
                            # members write DIFFERENT psum banks (cols 0:512
                            # vs 512:1024).
                            for pr in range(2):
                                for h in range(2):
                                    # h=0 (even j-chunk): parts 32:64
                                    # h=1 (odd j-chunk):  parts 0:32
                                    psl = slice((1 - h) * CQK, (2 - h) * CQK)
                                    col = (h * 512) + pr * 256
                                    nc.tensor.matmul(
                                        sp[:, col:col + 256],
                                        lhsT=k_sb[psl, s, 2 * jq + pr, :],
                                        rhs=q_sb[psl, s, i0:i0 + 256],
                                        start=True,
                                        stop=True,
                                    )
                            et = epool.tile([128, 1024], f8, tag="e")
                            exp_ctr += DVE_SHARE
                            if exp_ctr >= 256:
                                exp_ctr -= 256
                                # DVE fast-exp: uint8 bits ARE the fp8 e^s
                                nc.vector.tensor_scalar(
                                    et.bitcast(u8), sp, EK, EC, ALU.mult, ALU.add
                                )
                            else:
                                nc.scalar.activation(out=et, in_=sp, func=AF.Exp)
                            pend.append((s, ib, jq, et, ops4))
                            if len(pend) > PIPE:
                                ps_, pib_, pjq_, pet_, pops_ = pend.pop(0)
                                emit_out_mms(ps_, pops_, pjq_, pet_)
                                if pjq_ == JQ - 1:
                                    emit_normalize(ps_, pib_, pops_)
                for ps_, pib_, pjq_, pet_, pops_ in pend:
                    emit_out_mms(ps_, pops_, pjq_, pet_)
                    if pjq_ == JQ - 1:
                        emit_normalize(ps_, pib_, pops_)

    nc.compile()
    return nc


def kernel(x, Wq, bq, Wk, bk, Wv, bv, gamma):
    x = np.asarray(x, dtype=np.float32)
    Wq = np.asarray(Wq, dtype=np.float32)
    bq = np.asarray(bq, dtype=np.float32)
    Wk = np.asarray(Wk, dtype=np.float32)
    bk = np.asarray(bk, dtype=np.float32)
    Wv = np.asarray(Wv, dtype=np.float32)
    bv = np.asarray(bv, dtype=np.float32)
    g = float(np.asarray(gamma).reshape(-1)[0])

    xf = x.reshape(B, C, N)
    # residual with bv folded in, pre-transposed for contiguous DMA
    xr = np.ascontiguousarray(xf.transpose(0, 2, 1)) + (g * bv)[None, None, :]

    key = round(g, 12)
    if key not in _PROG_CACHE:
        _PROG_CACHE[key] = _build(g)
    nc = _PROG_CACHE[key]

    fp8 = ml_dtypes.float8_e4m3
    wqk = np.ascontiguousarray(
        np.concatenate([Wq.T, Wk.T], axis=1) * WSCALE
    ).astype(fp8)
    wvt = np.ascontiguousarray(Wv.T * WSCALE).astype(fp8)
    x_f8 = xf.astype(fp8)
    in_maps = []
    for core in range(NCORES):
        sl = slice(core * S, (core + 1) * S)
        in_maps.append(
            {
                "x": np.ascontiguousarray(x_f8[sl]),
                "xr": np.ascontiguousarray(xr[sl]),
                "wqk": wqk,
                "wvt": wvt,
                "bq": bq.reshape(CQK, 1),
                "bk": bk.reshape(CQK, 1),
            }
        )
    global last_results
    res = run_bass_kernel_spmd(nc, in_maps, list(range(NCORES)), trace=TRACE)
    last_results = res
    outs = []
    for core in range(NCORES):
        o = res.results[core]["o"]  # [S, N, C]
        outs.append(o.transpose(0, 2, 1))
    out = np.concatenate(outs, axis=0).reshape(B, C, HH, WW)
    return out.astype(np.float32)


if __name__ == "__main__":
    rng = np.random.default_rng(0)
    inputs = {
        "x": rng.standard_normal((B, C, HH, WW), dtype=np.float32),
        "Wq": rng.standard_normal((CQK, C), dtype=np.float32) * 0.02,
        "bq": rng.standard_normal((CQK,), dtype=np.float32) * 0.02,
        "Wk": rng.standard_normal((CQK, C), dtype=np.float32) * 0.02,
        "bk": rng.standard_normal((CQK,), dtype=np.float32) * 0.02,
        "Wv": rng.standard_normal((C, C), dtype=np.float32) * 0.02,
        "bv": rng.standard_normal((C,), dtype=np.float32) * 0.02,
        "gamma": rng.standard_normal((1,), dtype=np.float32) * 0.1,
    }
    out = kernel(**inputs)
    print("out", out.shape, out.dtype)
